# revision 6
# baseline (speedup 1.0000x reference)
r"""Lovasz hinge loss (nn_LovaszLoss) Trainium2 Bass kernel.

Math: per channel with errors e_i = 1 - logit_i * sign_i (sign = 2*label-1),
the loss equals the exact integral
    L = \int_0^inf N(t) / (G + M(t)) dt
where N(t) = #{i: e_i > t}, M(t) = #{negatives: e_i > t}, G = #positives.
Device computes, on a uniform grid t_k = k*delta (k = 0..K):
    R_N[k] = sum_i relu(e_i - t_k)          = \int_{t_k}^inf N dt   (exact)
    R_M[k] = sum_{neg} relu(e_i - t_k)      = \int_{t_k}^inf M dt   (exact)
so A[k] = R[k]-R[k+1] are exact per-bin integrals of N and M.  Then
    L_K = sum_k A_N[k] / (G + A_M[k]/delta)
has O(delta^2) error, and Richardson extrapolation with the half grid
(L* = (4 L_K - L_{K/2})/3, derived from the same R values) leaves ~2e-5
relative error at K=16 (validated in numpy, below the f32 reference's own
rounding noise).

Implementation: v = (t - 0.5) * x  (so e = 1 - 2v for both classes), fp16.
v_neg = v + 1024*t masks positives out of the M-family counts.
    relu(e - t_k) = 2*(c_k/2 - min(v, c_k/2)),  c_k = 1 - t_k
Family N runs on the Scalar engine: relu(-2*v + c_k) with fused accumulate.
Family M (and a few N thresholds, for engine balance) run on the Vector
engine: tensor_scalar(min, c_k/2) with fused accumulate.  Per-channel G is
recovered exactly from the difference of the v / v_neg pass accumulators.

Sharding: 64 channels, 8 per core, each channel 409600 elements laid out as
16 partitions x 25600.  Each core returns its 8 per-channel losses; the host
takes the mean of the 64 values.
"""

import numpy as np
from contextlib import ExitStack

import concourse.bass as bass
import concourse.bacc as bacc
import concourse.mybir as mybir
import concourse.tile as tile
from concourse.bass_utils import run_bass_kernel_spmd

F32 = mybir.dt.float32
F16 = mybir.dt.float16
I32 = mybir.dt.int32
Alu = mybir.AluOpType
Act = mybir.ActivationFunctionType

# ---- problem geometry (hardcoded per contract) ----
B, C, H, W = 16, 4, 256, 1600
NCH = B * C                    # 64 channels
NCORE = 8
CH_PER_CORE = NCH // NCORE     # 8
PSUB = 16                      # partitions per channel
P = CH_PER_CORE * PSUB         # 128
FD = (H * W) // PSUB           # 25600 elements per partition
CH_N = H * W                   # 409600 elements per channel

# ---- algorithm parameters ----
K = 16                         # number of bins (K+1 grid points); must be even
TMAX = 6.5
DELTA = TMAX / K
MASK = 1024.0                  # additive mask for positives in v_neg
NCHUNK = 8
FDC = FD // NCHUNK             # 3200
NK = K + 1
SPLIT = 11                     # thresholds k < SPLIT on ACT; k >= SPLIT on DVE


def build_program(fd=FD, nchunk=NCHUNK, split=SPLIT):
    fdc = fd // nchunk
    nc = bacc.Bacc(
        "TRN2", target_bir_lowering=False, debug=False, num_devices=NCORE
    )
    x_d = nc.dram_tensor("x", [P, fd], F32, kind="ExternalInput").ap()
    t_d = nc.dram_tensor("t", [P, fd], I32, kind="ExternalInput").ap()
    out_d = nc.dram_tensor("out", [CH_PER_CORE, 1], F32, kind="ExternalOutput").ap()

    tk = np.arange(NK) * DELTA
    ck = 1.0 - tk
    bias_np = np.tile(ck.astype(np.float32), (P, 1))            # [128, NK]
    chalf_np = np.tile((ck / 2).astype(np.float32), (P, 1))     # [128, NK]
    # epilogue corrections on st8 [8, 53]:
    #   cols 0..NK-1      : ACT N-family (direct R_N), alpha=1, beta=0; zero slots for k>=split
    #   cols NK..2NK-1    : DVE N-family min-form: R = F_ch*c_k - 2*acc (k>=split only)
    #   cols 2NK..3NK-1   : DVE M-family min-form: R = F_ch*c_k - 2*acc
    #   cols 3NK, 3NK+1   : sum(v), sum(v + MASK*t)
    WST = 3 * NK + 2
    alpha = np.zeros(WST, np.float32)
    beta = np.zeros(WST, np.float32)
    f_ch = float(fd * PSUB)
    for k in range(NK):
        alpha[k] = 1.0
        if k >= split:
            alpha[NK + k] = -2.0
            beta[NK + k] = f_ch * ck[k]
        alpha[2 * NK + k] = -2.0
        beta[2 * NK + k] = f_ch * ck[k]
    alpha[3 * NK] = 1.0
    alpha[3 * NK + 1] = 1.0
    alpha_np = np.tile(alpha, (CH_PER_CORE, 1))
    beta_np = np.tile(beta, (CH_PER_CORE, 1))

    bmask_np = np.zeros((P, CH_PER_CORE), np.float32)
    for p in range(P):
        bmask_np[p, p // PSUB] = 1.0
    bmask_h = nc.inline_tensor(bmask_np, "bmask")
    bias_h = nc.inline_tensor(bias_np, "biasN")
    chalf_h = nc.inline_tensor(chalf_np, "chalf")
    alpha_h = nc.inline_tensor(alpha_np, "alphac")
    beta_h = nc.inline_tensor(beta_np, "betac")

    with tile.TileContext(nc) as tc, ExitStack() as ctx:
        const_p = ctx.enter_context(tc.tile_pool(name="const", bufs=1))
        accs_p = ctx.enter_context(tc.tile_pool(name="accs", bufs=1))
        xst_p = ctx.enter_context(tc.tile_pool(name="xst", bufs=3))
        tst_p = ctx.enter_context(tc.tile_pool(name="tst", bufs=3))
        t16_p = ctx.enter_context(tc.tile_pool(name="t16", bufs=3))
        v_p = ctx.enter_context(tc.tile_pool(name="v", bufs=3))
        vn_p = ctx.enter_context(tc.tile_pool(name="vn", bufs=3))
        scra_p = ctx.enter_context(tc.tile_pool(name="scra", bufs=4))
        scrd_p = ctx.enter_context(tc.tile_pool(name="scrd", bufs=4))
        ep_p = ctx.enter_context(tc.tile_pool(name="ep", bufs=1))
        psum_p = ctx.enter_context(tc.tile_pool(name="psum", bufs=1, space="PSUM"))

        bias_t = const_p.tile([P, NK], F32, tag="bias")
        chalf_t = const_p.tile([P, NK], F32, tag="chalf")
        nc.sync.dma_start(bias_t[:], bias_h.ap())
        nc.sync.dma_start(chalf_t[:], chalf_h.ap())

        accNA = accs_p.tile([P, nchunk * NK], F32, tag="accNA")  # ACT N
        accND = accs_p.tile([P, nchunk * NK], F32, tag="accND")  # DVE N
        accM = accs_p.tile([P, nchunk * NK], F32, tag="accM")    # DVE M
        accA = accs_p.tile([P, nchunk], F32, tag="accA")
        accB = accs_p.tile([P, nchunk], F32, tag="accB")
        nc.vector.memset(accNA[:], 0.0)
        nc.vector.memset(accND[:], 0.0)

        for j in range(nchunk):
            sl = slice(j * fdc, (j + 1) * fdc)
            xt = xst_p.tile([P, fdc], F16, tag="xst")
            nc.gpsimd.dma_start(xt[:], x_d[:, sl])              # cast f32->f16
            tt32 = tst_p.tile([P, fdc], I32, tag="tst")
            nc.sync.dma_start(tt32[:], t_d[:, sl])
            tt16 = t16_p.tile([P, fdc], F16, tag="t16")
            nc.vector.tensor_copy(tt16[:], tt32[:])             # cast i32->f16

            vt = v_p.tile([P, fdc], F16, tag="v")
            nc.vector.scalar_tensor_tensor(
                vt[:], tt16[:], 0.5, xt[:],
                op0=Alu.subtract, op1=Alu.mult,
                accum_out=accA[:, j : j + 1],
            )
            vn = vn_p.tile([P, fdc], F16, tag="vn")
            nc.vector.scalar_tensor_tensor(
                vn[:], tt16[:], MASK, vt[:],
                op0=Alu.mult, op1=Alu.add,
                accum_out=accB[:, j : j + 1],
            )

            for k in range(NK):
                if k < split:
                    scr = scra_p.tile([P, fdc], F16, tag="scra")
                    nc.scalar.activation(
                        scr[:], vt[:], Act.Relu,
                        bias=bias_t[:, k : k + 1], scale=-2.0,
                        accum_out=accNA[:, j * NK + k : j * NK + k + 1],
                    )
                else:
                    scr = scrd_p.tile([P, fdc], F16, tag="scrd")
                    nc.vector.tensor_scalar(
                        scr[:], vt[:], chalf_t[:, k : k + 1], None,
                        op0=Alu.min, op1=Alu.add,
                        accum_out=accND[:, j * NK + k : j * NK + k + 1],
                    )
            for k in range(NK):
                scr = scrd_p.tile([P, fdc], F16, tag="scrd")
                nc.vector.tensor_scalar(
                    scr[:], vn[:], chalf_t[:, k : k + 1], None,
                    op0=Alu.min, op1=Alu.add,
                    accum_out=accM[:, j * NK + k : j * NK + k + 1],
                )

        # ---- epilogue ----
        S = ep_p.tile([P, WST], F32, tag="S")
        def chunk_sum(dst, acc, width):
            # acc: [P, nchunk*width], slot j at columns j*width..(j+1)*width
            nc.vector.tensor_tensor(
                dst, acc[:, 0:width], acc[:, width : 2 * width], op=Alu.add
            )
            for j in range(2, nchunk):
                nc.vector.tensor_tensor(
                    dst, dst, acc[:, j * width : (j + 1) * width], op=Alu.add
                )
        chunk_sum(S[:, 0:NK], accNA[:], NK)
        chunk_sum(S[:, NK : 2 * NK], accND[:], NK)
        chunk_sum(S[:, 2 * NK : 3 * NK], accM[:], NK)
        nc.vector.tensor_reduce(
            S[:, 3 * NK : 3 * NK + 1], accA[:], axis=mybir.AxisListType.X, op=Alu.add
        )
        nc.vector.tensor_reduce(
            S[:, 3 * NK + 1 : 3 * NK + 2], accB[:], axis=mybir.AxisListType.X, op=Alu.add
        )

        # 16->1 partition reduce per channel via PE: out[c, w] = sum_p mask[p, c] * S[p, w]
        bmask_t = const_p.tile([P, CH_PER_CORE], F32, tag="bmask")
        nc.sync.dma_start(bmask_t[:], bmask_h.ap())
        st8p = psum_p.tile([CH_PER_CORE, WST], F32, tag="st8p")
        nc.tensor.matmul(st8p[:], bmask_t[:], S[:], start=True, stop=True)
        st8 = ep_p.tile([CH_PER_CORE, WST], F32, tag="st8")
        nc.vector.tensor_copy(st8[:], st8p[:])

        alpha_t = ep_p.tile([CH_PER_CORE, WST], F32, tag="alpha")
        beta_t = ep_p.tile([CH_PER_CORE, WST], F32, tag="beta")
        nc.sync.dma_start(alpha_t[:], alpha_h.ap())
        nc.sync.dma_start(beta_t[:], beta_h.ap())
        stc = ep_p.tile([CH_PER_CORE, WST], F32, tag="stc")
        nc.vector.tensor_tensor(stc[:], st8[:], alpha_t[:], op=Alu.mult)
        nc.vector.tensor_tensor(stc[:], stc[:], beta_t[:], op=Alu.add)

        # R_N[k] = ACT part + DVE part; R_M from cols 2NK..3NK-1
        rn = ep_p.tile([CH_PER_CORE, NK], F32, tag="rn")
        nc.vector.tensor_tensor(rn[:], stc[:, 0:NK], stc[:, NK : 2 * NK], op=Alu.add)
        g_t = ep_p.tile([CH_PER_CORE, 1], F32, tag="g")
        nc.vector.tensor_tensor(
            g_t[:], stc[:, 3 * NK + 1 : 3 * NK + 2], stc[:, 3 * NK : 3 * NK + 1],
            op=Alu.subtract,
        )
        nc.vector.tensor_scalar(g_t[:], g_t[:], 1.0 / MASK, None, op0=Alu.mult)

        an = ep_p.tile([CH_PER_CORE, K], F32, tag="an")
        am = ep_p.tile([CH_PER_CORE, K], F32, tag="am")
        nc.vector.tensor_tensor(an[:], rn[:, 0:K], rn[:, 1:NK], op=Alu.subtract)
        nc.vector.tensor_tensor(
            am[:], stc[:, 2 * NK : 2 * NK + K], stc[:, 2 * NK + 1 : 3 * NK],
            op=Alu.subtract,
        )

        def grid_sum(a_n, a_m, nbins, delta, tag):
            den = ep_p.tile([CH_PER_CORE, nbins], F32, tag=tag + "d")
            nc.vector.tensor_scalar(
                den[:], a_m, 1.0 / delta, g_t[:], op0=Alu.mult, op1=Alu.add
            )
            # 1/den via exp(-ln(den)) + one Newton step (den >= G > 0)
            lnd = ep_p.tile([CH_PER_CORE, nbins], F32, tag=tag + "ln")
            nc.scalar.activation(lnd[:], den[:], Act.Ln)
            y0 = ep_p.tile([CH_PER_CORE, nbins], F32, tag=tag + "y0")
            nc.scalar.activation(y0[:], lnd[:], Act.Exp, scale=-1.0)
            dy = ep_p.tile([CH_PER_CORE, nbins], F32, tag=tag + "dy")
            nc.vector.tensor_tensor(dy[:], den[:], y0[:], op=Alu.mult)
            nc.vector.tensor_scalar(dy[:], dy[:], -1.0, 2.0, op0=Alu.mult, op1=Alu.add)
            rec = ep_p.tile([CH_PER_CORE, nbins], F32, tag=tag + "r")
            nc.vector.tensor_tensor(rec[:], y0[:], dy[:], op=Alu.mult)
            trm = ep_p.tile([CH_PER_CORE, nbins], F32, tag=tag + "t")
            nc.vector.tensor_tensor(trm[:], a_n, rec[:], op=Alu.mult)
            lsum = ep_p.tile([CH_PER_CORE, 1], F32, tag=tag + "s")
            nc.vector.tensor_reduce(
                lsum[:], trm[:], axis=mybir.AxisListType.X, op=Alu.add
            )
            return lsum

        l1 = grid_sum(an[:], am[:], K, DELTA, "l1")

        an2 = ep_p.tile([CH_PER_CORE, K // 2], F32, tag="an2")
        am2 = ep_p.tile([CH_PER_CORE, K // 2], F32, tag="am2")
        anv = an[:].rearrange("c (a b) -> c a b", b=2)
        amv = am[:].rearrange("c (a b) -> c a b", b=2)
        nc.vector.tensor_tensor(an2[:], anv[:, :, 0], anv[:, :, 1], op=Alu.add)
        nc.vector.tensor_tensor(am2[:], amv[:, :, 0], amv[:, :, 1], op=Alu.add)
        l2 = grid_sum(an2[:], am2[:], K // 2, 2 * DELTA, "l2")

        t1 = ep_p.tile([CH_PER_CORE, 1], F32, tag="t1")
        nc.vector.tensor_scalar(t1[:], l1[:], 4.0, None, op0=Alu.mult)
        nc.vector.tensor_tensor(t1[:], t1[:], l2[:], op=Alu.subtract)
        lstar = ep_p.tile([CH_PER_CORE, 1], F32, tag="lstar")
        nc.vector.tensor_scalar(lstar[:], t1[:], 1.0 / 3.0, None, op0=Alu.mult)
        nc.sync.dma_start(out_d[:], lstar[:])

    nc.compile()
    return nc


_CACHE = {}
LAST_EXEC_NS = [None]


def kernel(input, target):
    x = np.ascontiguousarray(np.asarray(input, dtype=np.float32))
    t = np.ascontiguousarray(np.asarray(target, dtype=np.int32))
    xl = x.reshape(NCH, CH_N)
    tl = t.reshape(NCH, CH_N)

    if "nc" not in _CACHE:
        _CACHE["nc"] = build_program()
    nc = _CACHE["nc"]

    in_maps = []
    for c in range(NCORE):
        c0 = c * CH_PER_CORE
        xs = xl[c0 : c0 + CH_PER_CORE].reshape(P, FD)
        ts = tl[c0 : c0 + CH_PER_CORE].reshape(P, FD)
        in_maps.append({"x": np.ascontiguousarray(xs), "t": np.ascontiguousarray(ts)})

    import os
    trace = bool(os.environ.get("LOVASZ_TRACE"))
    res = run_bass_kernel_spmd(
        nc, in_maps, core_ids=list(range(NCORE)), trace=trace
    )
    LAST_EXEC_NS[0] = res.exec_time_ns
    losses = np.concatenate([r["out"].reshape(-1) for r in res.results])
    return np.float32(losses.mean())
